# revision 1
# baseline (speedup 1.0000x reference)
"""AdaptiveMixing kernel for 8 Trainium2 NeuronCores.

Strategy (per sharding hint): data-parallel over the flattened (batch, h*w)
pixel axis -- all ops are pointwise per pixel. 20000 pixels -> 2500 per core.
Small weights (conv_w 16MB, proj_w) are replicated. No collectives needed.
Each core: conv param-gen matmul -> per-pixel group mixing -> LN+ReLU -> proj.
"""
import numpy as np
import jax
import jax.numpy as jnp
from functools import partial

try:
    jax.config.update('jax_compilation_cache_dir', '/tmp/jax_kernel_cache')
    jax.config.update('jax_persistent_cache_min_compile_time_secs', 0.5)
except Exception:
    pass

B, C, H, W = 2, 256, 100, 100
G, P = 4, 8
CG = C // G  # 64
EPS = 1e-5
Q = H * W            # 10000
NCORES = 8
N = B * Q            # 20000 flattened pixels
SH = N // NCORES     # 2500 pixels per core
CHUNK = 250          # pixel chunk per inner step (keeps param tensor small)


def _chunk_compute(carry, xs, conv_w, conv_b, ln_g, ln_b, proj_w, proj_b):
    bev_c, pts_c = xs  # (CHUNK, C), (CHUNK, P, C) -- pts_c is bf16
    # 1x1-conv parameter generator: (CHUNK, G*CG*CG); bf16 weights, f32 accum
    param = jnp.einsum('sc,oc->so', bev_c.astype(jnp.bfloat16), conv_w,
                       preferred_element_type=jnp.float32) + conv_b
    param = param.reshape(CHUNK, G, CG, CG)
    pts_g = pts_c.reshape(CHUNK, P, G, CG).transpose(0, 2, 1, 3)  # (CHUNK,G,P,CG)
    mixed = jnp.einsum('sgpc,sgcd->sgpd', pts_g,
                       param.astype(jnp.bfloat16),
                       preferred_element_type=jnp.float32)
    mu = mixed.mean(-1, keepdims=True)
    var = jnp.var(mixed, -1, keepdims=True)
    act = jax.nn.relu((mixed - mu) * jax.lax.rsqrt(var + EPS) * ln_g + ln_b)
    flat = act.reshape(CHUNK, G, P * CG)
    out = jnp.einsum('sgi,oi->sgo', flat, proj_w) + proj_b  # (CHUNK, G, CG)
    return carry, out.reshape(CHUNK, G * CG)


def _shard_fn(bev_s, pts_s, conv_w, conv_b, ln_g, ln_b, proj_w, proj_b):
    # bev_s: (SH, C)  pts_s: (SH, P, C)
    nchunk = SH // CHUNK
    bev_ch = bev_s.reshape(nchunk, CHUNK, C)
    pts_ch = pts_s.reshape(nchunk, CHUNK, P, C)
    f = partial(_chunk_compute, conv_w=conv_w, conv_b=conv_b,
                ln_g=ln_g, ln_b=ln_b, proj_w=proj_w, proj_b=proj_b)
    _, outs = jax.lax.scan(f, 0, (bev_ch, pts_ch))
    return outs.reshape(SH, G * CG)


_pmapped = None


def _get_pmapped():
    global _pmapped
    if _pmapped is None:
        _pmapped = jax.pmap(
            _shard_fn, axis_name='i',
            in_axes=(0, 0, None, None, None, None, None, None),
            devices=jax.devices()[:NCORES])
    return _pmapped


def kernel(**inputs):
    bev = np.asarray(inputs['bev_query'], dtype=np.float32)
    pts = np.asarray(inputs['pts'], dtype=np.float32)
    conv_w = np.asarray(inputs['conv_w'], dtype=np.float32)
    conv_b = np.asarray(inputs['conv_b'], dtype=np.float32)
    ln_g = np.asarray(inputs['ln_g'], dtype=np.float32)
    ln_b = np.asarray(inputs['ln_b'], dtype=np.float32)
    proj_w = np.asarray(inputs['proj_w'], dtype=np.float32)
    proj_b = np.asarray(inputs['proj_b'], dtype=np.float32)

    # Shard: flatten (b, q) -> pixel axis, split across 8 cores.
    bev_p = bev.reshape(B, C, Q).transpose(0, 2, 1).reshape(NCORES, SH, C)
    pts_p = pts.reshape(B, Q, P, C).reshape(NCORES, SH, P, C)

    try:
        import ml_dtypes
        bf16 = ml_dtypes.bfloat16
        fn = _get_pmapped()
        out_sh = fn(jnp.asarray(bev_p),
                    jnp.asarray(pts_p.astype(bf16)),
                    jnp.asarray(conv_w.astype(bf16)),
                    jnp.asarray(conv_b),
                    jnp.asarray(ln_g), jnp.asarray(ln_b),
                    jnp.asarray(proj_w), jnp.asarray(proj_b))
        out = np.asarray(out_sh)  # (8, SH, 256)
    except Exception:
        # Host fallback (correctness safety net).
        out = np.empty((NCORES, SH, G * CG), dtype=np.float32)
        for i in range(NCORES):
            bev_s, pts_s = bev_p[i], pts_p[i]
            param = (bev_s @ conv_w.T + conv_b).reshape(SH, G, CG, CG)
            pts_g = pts_s.reshape(SH, P, G, CG).transpose(0, 2, 1, 3)
            mixed = np.einsum('sgpc,sgcd->sgpd', pts_g, param)
            mu = mixed.mean(-1, keepdims=True)
            var = mixed.var(-1, keepdims=True)
            act = np.maximum((mixed - mu) / np.sqrt(var + EPS) * ln_g + ln_b, 0.0)
            flat = act.reshape(SH, G, P * CG)
            out[i] = (np.einsum('sgi,oi->sgo', flat, proj_w)
                      + proj_b).reshape(SH, G * CG)

    # Unshard: (8, SH, 256) -> (B, 256, H, W)
    full = out.reshape(B, Q, G * CG).transpose(0, 2, 1).reshape(B, G * CG, H, W)
    return np.ascontiguousarray(full.astype(np.float32))

